# revision 70
# baseline (speedup 1.0000x reference)
"""DiagBlockAttention Trainium2 kernel (v2).

Full module: qkv = x @ w_qkv.T; block-diagonal attention over 16-token
groups (4x4 tiles of the (8, 512) token grid); out = attn_out @ w_out.T + b_out.

Sharding: data-parallel over batch -- batch element i runs on NeuronCore i
(no collectives).  All heavy matmuls run in bf16 on the TensorEngine with
fp32 PSUM accumulation (rel err vs fp32 oracle ~4e-3).

Per-core dataflow (x_b: [4096, 512] fp32):
  1. x -> SBUF, transpose to xT [512_fi, 4096_tok] via regular bf16
     identity matmuls.  The 4x4-block token permutation that makes each
     16-token attention group contiguous is folded into the free-dim
     access pattern of the PSUM->SBUF eviction.
  2. GEMM1a: qkT = W_qk-chunks.T @ xT.  Evicted ONE HEAD PER 512-col
     BLOCK into qk_sb: even heads occupy rows 0..64, odd heads rows
     64..128 (partition-aligned with their PSUM half).  The unused
     64-row half of every block holds a PRELOADED constant: 8 rows of
     16*one-hot(group) ("mask rows") + 56 zero rows.  This folds the
     block-diagonal attention mask into the S matmul contraction:
     S' = k'.T @ q' = k.T q + 256*same_group(k,q), with K=128.
     GEMM1b: v = xT-chunks.T @ W_v token-major, with a ones-column per
     head (fused softmax denominator).
  3. Attention per (128-token tile, head): ONE self-closed K=128 matmul
     S' = kblk.T @ qblk into psS (these pipeline back-to-back on the PE
     with LDWEIGHTS hidden, unlike accumulation groups which serialize);
     P = exp(S'/8 - 32) on ScalarE (off-group -> e^-32 ~ 0, in-group
     offset 256/8-32 cancels).  One matmul per head computes
     [O' | denom] = P.T @ [v | 1]; normalize+evict via a broadcasted
     reciprocal multiply on VectorE.
  4. O -> O^T per token-tile right after its normalize (spreads the
     transpose matmuls through the attention phase) via regular bf16
     matmuls against a PERMUTATION matrix that reorders token columns
     (mb,a,e)->(a,mb,e); GEMM2 (pipelined one iteration behind, emitted
     at the TOP of the next supertile so its bias-adds sit early in the
     DVE FIFO): final = O^T-chunks.T @ W_out + b_out comes out with
     partitions already in natural y-row order, so each fin tile is
     stored with a SINGLE gpsimd (SWDGE) DMA of four contiguous 64KB
     runs.  (v1 issued 256 small stores from the sync sequencer at
     565ns each -- SP was the end-of-kernel bottleneck.)

  Other scheduling notes: GEMM1a's two half-evictions per F-group go to
  ONE engine (alternating per F) so concurrent engines never read the
  same PSUM bank (same-bank dual reads serialize ~700ns); const/pad DMAs
  issue from the ACT HWDGE queue-set so the x loads (split across the
  SP and ACT queue-sets) start immediately; x rows 0..3 use tt-outer
  transpose emission so each arriving load feeds 4 matmuls at once.

Hardware notes baked into the structure (found by bisection on trn2):
  - a PSUM accumulation group whose matmuls interleave with another open
    group, or certain K=64 single-matmul groups at base_partition 64,
    fault the exec unit.  All attention matmuls here are K=128
    single-matmul closed groups at base_partition 0, which are safe AND
    pipeline (measured 32ns/MM for the O matmuls in the v1 trace);
  - LDWEIGHTS does not overlap its paired MATMUL across accumulation-
    group boundaries on this toolchain, but does within closed-group
    streams (walrus --enable-ldw-opt is broken).
"""

import os
import sys
from contextlib import ExitStack

sys.path.insert(0, "/opt/trn_rl_repo")

import ml_dtypes
import numpy as np

import concourse.bass as bass
import concourse.mybir as mybir
import concourse.tile as tile
from concourse import bacc
from concourse.bass_utils import run_bass_kernel_spmd


def _ensure_ntff_hook():
    """This image's antenv lacks axon_hooks; synthesize it so trace=True
    (NTFF profiling) works through run_bass_kernel_spmd."""
    import types

    try:
        from antenv import axon_hooks  # noqa: F401
        return
    except ImportError:
        pass
    try:
        import antenv
        from trn_agent_boot.trn_boot import _ntff_profile_via_ctypes

        mod = types.ModuleType("antenv.axon_hooks")
        _hook = [None]
        mod.set_axon_ntff_profile_hook = lambda h: _hook.__setitem__(0, h)
        mod.get_axon_ntff_profile_hook = lambda: _hook[0]
        sys.modules["antenv.axon_hooks"] = mod
        antenv.axon_hooks = mod
        mod.set_axon_ntff_profile_hook(
            _ntff_profile_via_ctypes("/opt/axon/libaxon_pjrt.so"))
    except Exception as e:  # pragma: no cover
        print(f"ntff hook shim failed ({e}); tracing disabled", file=sys.stderr)


_ensure_ntff_hook()

F32 = mybir.dt.float32
BF16 = mybir.dt.bfloat16

N_CORES = 8
NT = 4096          # tokens per core
DIM = 512          # model dim
INNER = 512        # heads * dim_head
HEADS = 8
DH = 64            # dim head
NSUP = NT // 512   # 512-token supertiles
P = 128

SCALE = DH ** -0.5

QK_BUFS = 3
WARM_P0 = int(os.environ.get("KERNEL_WARM_P0", "0"))
WARM_OT = int(os.environ.get("KERNEL_WARM_OT", "0"))
WARM_ATT = int(os.environ.get("KERNEL_WARM_ATT", "0"))
# "gps" = single 4D store per fin tile issued from the gpsimd SWDGE;
# "sync" = v1-style 8 small stores per fin tile from the sync sequencer.
YDMA = os.environ.get("KERNEL_YDMA", "gps")


def build_kernel(zero_bias=False):
    nc = bacc.Bacc("TRN2", target_bir_lowering=False, debug=False)

    # x is pre-transposed + bf16-cast + token-grouped on HOST (same as the
    # weights): chunk (T, c) = xtg[T*4+c] is a CONTIGUOUS [128, 512] block
    # (feature chunk c, supertile T) so each load is one linear 128KB run.
    xtg = nc.dram_tensor("xtg", [NSUP * 4, P, 512], BF16,
                         kind="ExternalInput").ap()
    # per-tile contiguous weight blocks (chunk c on rows); wqkc is further
    # blocked per F-column-block so the first GEMM group's weights arrive
    # within ~2us: wqkc[c, F] = [128, 128] contiguous.
    wqkc = nc.dram_tensor("wqkc", [4, 8, P, P], BF16, kind="ExternalInput").ap()
    wvc = nc.dram_tensor("wvc", [4, P, 512], BF16, kind="ExternalInput").ap()
    woc = nc.dram_tensor("woc", [4, P, 512], BF16, kind="ExternalInput").ap()
    biasb = nc.dram_tensor("biasb", [P, DIM], F32, kind="ExternalInput").ap()
    permc = nc.dram_tensor("permc", [P, P], BF16, kind="ExternalInput").ap()
    # [8, 8*512]: 16*one-hot(group-of-16) mask rows, tiled across 8 blocks.
    # Loaded into the unused halves of the qk one-head-per-block tiles; the
    # zero rows below/above them are memset on-chip (was 3MB of DMA'd zeros).
    padc = nc.dram_tensor("padc", [8, 8 * 512], BF16, kind="ExternalInput").ap()
    y = nc.dram_tensor("y", [NT, DIM], F32, kind="ExternalOutput").ap()

    # store-side view: the O^T transpose uses a PERMUTATION matrix instead of
    # identity, so fin comes out of GEMM2 with partitions ordered (a, mb, e) =
    # natural y-row order; per (nb, a) a fin tile is one contiguous 64KB run.
    ya = y.rearrange("(nb a rw) f -> nb a (rw f)", nb=2, a=4, rw=512)

    with TileKernel(nc) as tc:
        ctx = tc._ctx
        const = ctx.enter_context(tc.tile_pool(name="const", bufs=1))
        xt_pool = ctx.enter_context(tc.tile_pool(name="xt", bufs=1))
        xload = ctx.enter_context(tc.tile_pool(name="xload", bufs=8))
        xbpool = ctx.enter_context(tc.tile_pool(name="xb", bufs=16))
        v_pool = ctx.enter_context(tc.tile_pool(name="v", bufs=3))
        p_pool = ctx.enter_context(tc.tile_pool(name="p", bufs=4))
        r_pool = ctx.enter_context(tc.tile_pool(name="r", bufs=24))
        o_pool = ctx.enter_context(tc.tile_pool(name="o", bufs=3))
        ot_pool = ctx.enter_context(tc.tile_pool(name="ot", bufs=3))
        f_pool = ctx.enter_context(tc.tile_pool(name="f", bufs=10))
        psum = ctx.enter_context(tc.tile_pool(name="psum", bufs=2, space="PSUM"))

        # --- PE warm-up: ~50 dummy N=128 matmuls on a memset tile, runnable
        # the moment the preamble ends.  Gets the HAM clock gate to 8/8
        # (2.4 GHz) before the first real GEMM group issues. ---
        warmw = const.tile([P, P], BF16, tag="warmw")
        nc.vector.memset(warmw[:], 0.0)
        ps_warm = psum.tile([P, P], F32, name="ps_warm", tag="att", bufs=3)
        for _ in range(100):
            nc.tensor.matmul(ps_warm[:, 0:64], warmw[:], warmw[:, 0:64],
                             start=True, stop=True)

        # --- constants / weights (GEMM1 weights first: they gate GEMM1a(0)).
        # Early loads are split into partition-quarter DMAs round-robined
        # over the three issue paths so they spread across many queues. ---
        # HWDGE only: gpsimd SWDGE pays ~620ns of descriptor-gen per DMA on
        # the gpsimd engine and serializes -- keep it off the critical path.
        ld_engs = [nc.sync, nc.scalar]
        ld_i = [0]

        def fast_load(dst, src, ways=4):
            n = dst.shape[0]
            step = n // ways
            for qq in range(ways):
                ld_engs[ld_i[0] % len(ld_engs)].dma_start(
                    dst[qq * step:(qq + 1) * step], src[qq * step:(qq + 1) * step])
                ld_i[0] += 1

        wqk = []
        wv = []
        wo = []
        for c in range(4):
            t = const.tile([P, 1024], BF16, tag=f"wqk{c}")
            wqk.append(t)
        # F-major interleave: the F0 blocks of all four chunks land first,
        # so GEMM1a(0)'s first group can start ~2us after the preamble.
        for F in range(8):
            for c in range(4):
                ld_engs[ld_i[0] % len(ld_engs)].dma_start(
                    wqk[c][:, F * P:(F + 1) * P], wqkc[c, F])
                ld_i[0] += 1

        expb = const.tile([P, 1], F32, tag="expb")
        nc.vector.memset(expb[:], -32.0)

        # --- x load: xtg is already transposed/grouped/bf16 (host-prepped);
        # T=0/1 chunks issue right after the GEMM1 weights so supertile 0
        # can start within a few us; the remaining consts and supertiles
        # follow.  Spread across the sync/scalar HWDGE + gpsimd SWDGE
        # queue-sets. ---
        xT = xt_pool.tile([P, 4 * NT], BF16)  # chunk c at cols [c*NT, ...)

        def load_xt(T, ways=1):
            for c in range(4):
                fast_load(
                    xT[:, c * NT + T * 512: c * NT + (T + 1) * 512],
                    xtg[T * 4 + c], ways=ways)

        load_xt(0, ways=2)

        # qk buffers: even-head blocks put data at rows 0:64 + mask at 64:72
        # and the S matmuls contract only K=72 rows, so rows 72:128 stay
        # uninitialized-but-unread (no memset).  Odd-head blocks have data at
        # 64:128 + mask at 0:8 with K=128, so rows 8:64 must be zeroed once.
        # Mask DMAs are tiny (64KB) and emitted early: attention(0) needs
        # them at ~15us and they must not queue behind bulk loads.
        qk_tiles = []
        for i in range(QK_BUFS):
            t = const.tile([P, 16 * 512], BF16, tag=f"qkbuf{i}")
            # all on gpsimd: V must be free for the first GEMM1a evictions
            nc.gpsimd.memset(t[0:64, 8 * 512:16 * 512], 0.0)
            nc.scalar.dma_start(t[64:72, 0:8 * 512], padc[:])
            nc.sync.dma_start(t[0:8, 8 * 512:16 * 512], padc[:])
            qk_tiles.append(t)

        for c in range(4):
            t = const.tile([P, 512], BF16, tag=f"wv{c}")
            fast_load(t[:], wvc[c], ways=2)
            wv.append(t)
        load_xt(1, ways=2)

        for c in range(4):
            t = const.tile([P, 512], BF16, tag=f"wo{c}")
            nc.scalar.dma_start(t[:], woc[c])
            wo.append(t)
        permb = const.tile([P, P], BF16, tag="permb")
        nc.sync.dma_start(permb[:], permc[:])
        bias = const.tile([P, DIM], F32, tag="bias")
        nc.scalar.dma_start(bias[:], biasb[:])
        # Fixed qk double... triple-buffer with pad halves written ONCE.
        # (Fixed const tiles instead of a rotating pool so the preloaded
        # mask+zero rows deterministically persist across supertiles.)
        # Block layout (even/odd-contiguous so the two pad DMAs are plain
        # contiguous writes): even heads h -> k block h/2, q block 4+h/2
        # (data rows 0..64, pad rows 64..128); odd heads h -> k block 8+h/2,
        # q block 12+h/2 (data rows 64..128, pad rows 0..64).
        # bulk loads stay OFF the scalar (ACT) queue: 20+ queued DMA-issue
        # ops fill the ring and head-block ACT's instruction FIFO, stalling
        # the GEMM1a evictions behind them.
        xt_rest_engs = [nc.sync, nc.gpsimd]
        for T in range(2, NSUP):
            for c in range(4):
                xt_rest_engs[(T * 4 + c) % 2].dma_start(
                    xT[:, c * NT + T * 512: c * NT + (T + 1) * 512],
                    xtg[T * 4 + c],
                )

        st_engs = [nc.gpsimd, nc.sync, nc.scalar]

        def emit_g2_tt(gT, g_ot, tt):
            ps = psum.tile([P, 512], F32, name="ps_g2", tag="aux", bufs=2)
            for c in range(4):
                nc.tensor.matmul(
                    ps[:],
                    g_ot[:, tt * 512 + c * P: tt * 512 + (c + 1) * P],
                    wo[c][:],
                    start=(c == 0),
                    stop=(c == 3),
                )
            fin = f_pool.tile([P, DIM], F32, name="fin")
            if zero_bias:
                # b_out == 0: plain eviction on ACT keeps the busier DVE free
                nc.scalar.copy(fin[:], ps[:])
            else:
                nc.vector.tensor_tensor(
                    fin[:], ps[:], bias[:], op=mybir.AluOpType.add)
            t_idx = gT * 4 + tt
            nb, ms = t_idx // 16, 8 * (t_idx % 16)
            # fin partitions are (a, mb, e) = natural order; per a the 32
            # rows land in one contiguous 64KB run of y.  Four separate
            # DMAs on rotating queue-sets so the drain parallelizes.
            for a in range(4):
                st_engs[(t_idx * 4 + a) % 3].dma_start(
                    ya[nb, a, ms * 2048: ms * 2048 + 16384],
                    fin[a * 32:(a + 1) * 32, :],
                )

        def emit_g2(gT, g_ot):
            for tt in range(4):
                emit_g2_tt(gT, g_ot, tt)

        prev_ot = None
        # --- main loop over 512-token supertiles ---
        # emit_g2(T-1) is emitted FIRST in iteration T: its DVE bias-adds
        # then precede iteration T's evictions in the engine FIFOs, and its
        # matmuls are immediately runnable -- emitting it after T's body put
        # a blocked fin-bias at the DVE queue head and convoyed the PE.
        for T in range(NSUP):
            if T > 0:
                emit_g2(T - 1, prev_ot)
            # qk one-head-per-block layout: block b (512 cols) holds head
            # features in one 64-row half, preloaded mask+zero rows in the
            # other.  k head h -> block h; q head h -> block 8+h.
            # Even h: data rows 0..64 (pad rows 64..128);
            # odd  h: data rows 64..128 (pad rows 0..64).
            qk_sb = qk_tiles[T % QK_BUFS]

            # GEMM1a: qkT [1024_fo, 512_tok] -> bf16, one head per block.
            # BOTH half-evictions of a group go to ONE engine (alternating
            # per F): concurrent engines then always read different PSUM
            # banks (same-bank dual reads serialize ~700ns).
            for F in range(8):
                ps = psum.tile([P, 512], F32, name="ps_g1a", tag="g1a", bufs=3)
                for c in range(4):
                    nc.tensor.matmul(
                        ps[:],
                        wqk[c][:, F * P:(F + 1) * P],
                        xT[:, c * NT + T * 512: c * NT + (T + 1) * 512],
                        start=(c == 0),
                        stop=(c == 3),
                    )
                # F 0..3 = q heads (2F, 2F+1); F 4..7 = k heads (2(F-4), ...)
                if F < 4:
                    be, bo = 4 + F, 12 + F
                else:
                    be, bo = F - 4, 8 + (F - 4)
                dst_e = qk_sb[0:64, be * 512:(be + 1) * 512]
                dst_o = qk_sb[64:128, bo * 512:(bo + 1) * 512]
                if F % 2 == 0:
                    nc.vector.tensor_copy(dst_e, ps[0:64, :])
                    nc.vector.tensor_copy(dst_o, ps[64:128, :])
                else:
                    nc.scalar.copy(dst_e, ps[0:64, :])
                    nc.scalar.copy(dst_o, ps[64:128, :])

            # GEMM1b: v [512_tok, 512_fo] -> bf16 (token-major), 66-stride
            # per head with a ones-column at offset 64 (fused denominator)
            v_sb = v_pool.tile([P, 4 * 528], BF16, name="v_sb")
            vview = v_sb[:].rearrange("p (tt h e) -> p tt h e", tt=4, h=8)
            nc.vector.memset(vview[:, :, :, 64:65], 1.0)
            for tt in range(4):
                ps = psum.tile([P, 512], F32, name="ps_g1b", tag="aux", bufs=2)
                for c in range(4):
                    nc.tensor.matmul(
                        ps[:],
                        xT[:, c * NT + T * 512 + tt * P:
                           c * NT + T * 512 + (tt + 1) * P],
                        wv[c][:],
                        start=(c == 0),
                        stop=(c == 3),
                    )
                vdst = vview[:, tt, :, 0:64]
                nc.vector.tensor_copy(
                    vdst, ps[:].rearrange("p (h d) -> p h d", h=8))

            # attention: tt outer, head-groups of 4 inner
            o_sb = o_pool.tile([P, 4 * 512], BF16)
            ot_sb = ot_pool.tile([P, 4 * 512], BF16)
            for tt in range(4):
                psAs = []
                for hg in range(2):
                    # one PSUM bank per (tt, hg) chain: S writes [P,512],
                    # exp consumes it, then O reuses cols 0:288 of the SAME
                    # bank (frees banks for deeper GEMM pipelining).
                    psA = psum.tile([P, 512], F32, name="psA", tag="att",
                                    bufs=3)
                    psAs.append(psA)
                    for hh in range(4):
                        h = hg * 4 + hh
                        kb = h // 2 if h % 2 == 0 else 8 + h // 2
                        qb = 4 + h // 2 if h % 2 == 0 else 12 + h // 2
                        # even heads: data rows 0:64 + mask 64:72 -> K=72
                        # (rows 72:128 never initialized); odd heads: K=128.
                        kp = 72 if h % 2 == 0 else P
                        ksl = qk_sb[0:kp, kb * 512 + tt * P:
                                    kb * 512 + (tt + 1) * P]
                        qsl = qk_sb[0:kp, qb * 512 + tt * P:
                                    qb * 512 + (tt + 1) * P]
                        nc.tensor.matmul(
                            psA[:, hh * P:(hh + 1) * P], ksl, qsl,
                            start=True, stop=True,
                        )
                    p_sb = p_pool.tile([P, 512], BF16)
                    nc.scalar.activation(
                        p_sb[:], psA[:],
                        mybir.ActivationFunctionType.Exp,
                        bias=expb[:], scale=SCALE,
                    )
                    for hh in range(4):
                        h = hg * 4 + hh
                        psl = p_sb[:, hh * P:(hh + 1) * P]
                        nc.tensor.matmul(
                            psA[:, hh * 72: hh * 72 + 65],
                            psl,
                            v_sb[:, tt * 528 + h * 66:
                                 tt * 528 + h * 66 + 65],
                            start=True, stop=True,
                        )

                # normalize + evict O (token-major bf16)
                for hg in range(2):
                    rc = r_pool.tile([P, 4], F32, name="rc")
                    dsl = psAs[hg][:, 0:288].rearrange("p (h e) -> p h e", h=4)
                    nc.vector.reciprocal(rc[:], dsl[:, :, 64:65].squeeze(2))
                    dst = o_sb[:, tt * 512 + hg * 256:
                               tt * 512 + (hg + 1) * 256].rearrange(
                        "p (h d) -> p h d", h=4)
                    rbc = rc[:].unsqueeze(2).broadcast_to([P, 4, DH])
                    nc.vector.tensor_tensor(
                        dst, dsl[:, :, 0:64], rbc, op=mybir.AluOpType.mult)

                # O^T for this tt (needs only this tt's normalize): regular
                # bf16 matmul lhsT.T @ Perm -- Perm reorders token columns
                # (mb,a,e)->(a,mb,e) so GEMM2's output partitions are already
                # in natural y-row order.  Emitting per-tt spreads the
                # transpose matmuls and their evictions through the attention
                # phase instead of bunching them at the supertile boundary.
                ps_ot = psum.tile([P, 512], F32, name="ps_ot", tag="aux",
                                  bufs=2)
                for c in range(4):
                    nc.tensor.matmul(
                        ps_ot[:, c * P:(c + 1) * P],
                        o_sb[:, tt * 512 + c * P: tt * 512 + (c + 1) * P],
                        permb[:],
                        start=True, stop=True,
                    )
                if tt % 2 == 0:
                    nc.scalar.copy(
                        ot_sb[:, tt * 512:(tt + 1) * 512], ps_ot[:])
                else:
                    nc.vector.tensor_copy(
                        ot_sb[:, tt * 512:(tt + 1) * 512], ps_ot[:])
                if T == NSUP - 1:
                    # final supertile: emit its GEMM2 per-tt right behind the
                    # ot eviction so the tail is one tt deep, not four.
                    emit_g2_tt(T, ot_sb, tt)

            prev_ot = ot_sb

    nc.compile()
    return nc


class TileKernel:
    """TileContext wrapper that also owns an ExitStack for pools."""

    def __init__(self, nc):
        self.nc = nc
        self._tc = tile.TileContext(nc)
        self._ctx = ExitStack()

    def __enter__(self):
        tc = self._tc.__enter__()
        tc._ctx = self._ctx
        return tc

    def __exit__(self, *exc):
        self._ctx.close()
        return self._tc.__exit__(*exc)


def _host_inputs(x, w_qkv, w_out, b_out):
    # x -> xT in grouped token order, bf16: token (nb, a, mb, e) lands at
    # grouped col nb*2048 + mb*16 + a*4 + e; feature f on rows.
    b = x.shape[0]
    xtg = (
        x.reshape(b, 2, 4, 128, 4, DIM)          # b, nb, a, mb, e, f
        .transpose(0, 5, 1, 3, 2, 4)             # b, f, nb, mb, a, e
        .reshape(b, DIM, NT)
        .astype(ml_dtypes.bfloat16)
    )
    # chunk (T, c) contiguous: [b, 32, 128, 512] with index T*4+c
    xtg = np.ascontiguousarray(
        xtg.reshape(b, 4, P, NSUP, 512).transpose(0, 3, 1, 2, 4)
        .reshape(b, NSUP * 4, P, 512))
    wqkvT = np.ascontiguousarray(w_qkv.T).astype(ml_dtypes.bfloat16)
    woutT = np.ascontiguousarray(w_out.T).astype(ml_dtypes.bfloat16)
    wqkc = np.ascontiguousarray(
        wqkvT[:, 0:1024].reshape(4, P, 8, P).transpose(0, 2, 1, 3))
    wvc = np.ascontiguousarray(
        wqkvT[:, 1024:1536].reshape(4, P, 512))
    woc = np.ascontiguousarray(woutT.reshape(4, P, 512))
    biasb = np.ascontiguousarray(
        np.broadcast_to(b_out.astype(np.float32), (P, DIM)))
    # permutation: out col j = a*32+mb*4+e reads token p = mb*16+a*4+e
    permc = np.zeros((P, P), dtype=np.float32)
    for mb in range(8):
        for a in range(4):
            for e in range(4):
                permc[mb * 16 + a * 4 + e, a * 32 + mb * 4 + e] = 1.0
    permc = permc.astype(ml_dtypes.bfloat16)
    # pad block: 16*one-hot(group-of-16 of col%128) mask rows only
    pad1 = np.zeros((8, 512), dtype=np.float32)
    for g in range(8):
        for rep in range(4):
            pad1[g, rep * 128 + g * 16: rep * 128 + (g + 1) * 16] = 16.0
    padc = np.tile(pad1, (1, 8)).astype(ml_dtypes.bfloat16)
    shared = {
        "wqkc": wqkc, "wvc": wvc, "woc": woc, "biasb": biasb,
        "permc": np.ascontiguousarray(permc),
        "padc": np.ascontiguousarray(padc),
    }
    return [
        {"xtg": np.ascontiguousarray(xtg[i]), **shared}
        for i in range(N_CORES)
    ]


_NC_CACHE = {}


def _get_nc(zero_bias):
    key = ("nc", zero_bias)
    if key not in _NC_CACHE:
        _NC_CACHE[key] = build_kernel(zero_bias=zero_bias)
    return _NC_CACHE[key]


def kernel(x, w_qkv, w_out, b_out, _trace=False, _trace_kwargs=None):
    nc = _get_nc(bool(np.all(np.asarray(b_out) == 0.0)))
    in_maps = _host_inputs(x, w_qkv, w_out, b_out)
    kw = {}
    if _trace:
        kw = dict(trace=True, **(_trace_kwargs or {}))
    res = run_bass_kernel_spmd(nc, in_maps, core_ids=list(range(N_CORES)), **kw)
    out = np.stack([res.results[i]["y"] for i in range(N_CORES)], axis=0)
    if _trace:
        kernel.last_results = res
    return out



# revision 72
# speedup vs baseline: 1.0076x; 1.0076x over previous
"""DiagBlockAttention Trainium2 kernel (v2).

Full module: qkv = x @ w_qkv.T; block-diagonal attention over 16-token
groups (4x4 tiles of the (8, 512) token grid); out = attn_out @ w_out.T + b_out.

Sharding: data-parallel over batch -- batch element i runs on NeuronCore i
(no collectives).  All heavy matmuls run in bf16 on the TensorEngine with
fp32 PSUM accumulation (rel err vs fp32 oracle ~4e-3).

Per-core dataflow (x_b: [4096, 512] fp32):
  1. x -> SBUF, transpose to xT [512_fi, 4096_tok] via regular bf16
     identity matmuls.  The 4x4-block token permutation that makes each
     16-token attention group contiguous is folded into the free-dim
     access pattern of the PSUM->SBUF eviction.
  2. GEMM1a: qkT = W_qk-chunks.T @ xT.  Evicted ONE HEAD PER 512-col
     BLOCK into qk_sb: even heads occupy rows 0..64, odd heads rows
     64..128 (partition-aligned with their PSUM half).  The unused
     64-row half of every block holds a PRELOADED constant: 8 rows of
     16*one-hot(group) ("mask rows") + 56 zero rows.  This folds the
     block-diagonal attention mask into the S matmul contraction:
     S' = k'.T @ q' = k.T q + 256*same_group(k,q), with K=128.
     GEMM1b: v = xT-chunks.T @ W_v token-major, with a ones-column per
     head (fused softmax denominator).
  3. Attention per (128-token tile, head): ONE self-closed K=128 matmul
     S' = kblk.T @ qblk into psS (these pipeline back-to-back on the PE
     with LDWEIGHTS hidden, unlike accumulation groups which serialize);
     P = exp(S'/8 - 32) on ScalarE (off-group -> e^-32 ~ 0, in-group
     offset 256/8-32 cancels).  One matmul per head computes
     [O' | denom] = P.T @ [v | 1]; normalize+evict via a broadcasted
     reciprocal multiply on VectorE.
  4. O -> O^T per token-tile right after its normalize (spreads the
     transpose matmuls through the attention phase) via regular bf16
     matmuls against a PERMUTATION matrix that reorders token columns
     (mb,a,e)->(a,mb,e); GEMM2 (pipelined one iteration behind, emitted
     at the TOP of the next supertile so its bias-adds sit early in the
     DVE FIFO): final = O^T-chunks.T @ W_out + b_out comes out with
     partitions already in natural y-row order, so each fin tile is
     stored with a SINGLE gpsimd (SWDGE) DMA of four contiguous 64KB
     runs.  (v1 issued 256 small stores from the sync sequencer at
     565ns each -- SP was the end-of-kernel bottleneck.)

  Other scheduling notes: GEMM1a's two half-evictions per F-group go to
  ONE engine (alternating per F) so concurrent engines never read the
  same PSUM bank (same-bank dual reads serialize ~700ns); const/pad DMAs
  issue from the ACT HWDGE queue-set so the x loads (split across the
  SP and ACT queue-sets) start immediately; x rows 0..3 use tt-outer
  transpose emission so each arriving load feeds 4 matmuls at once.

Hardware notes baked into the structure (found by bisection on trn2):
  - a PSUM accumulation group whose matmuls interleave with another open
    group, or certain K=64 single-matmul groups at base_partition 64,
    fault the exec unit.  All attention matmuls here are K=128
    single-matmul closed groups at base_partition 0, which are safe AND
    pipeline (measured 32ns/MM for the O matmuls in the v1 trace);
  - LDWEIGHTS does not overlap its paired MATMUL across accumulation-
    group boundaries on this toolchain, but does within closed-group
    streams (walrus --enable-ldw-opt is broken).
"""

import os
import sys
from contextlib import ExitStack

sys.path.insert(0, "/opt/trn_rl_repo")

import ml_dtypes
import numpy as np

import concourse.bass as bass
import concourse.mybir as mybir
import concourse.tile as tile
from concourse import bacc
from concourse.bass_utils import run_bass_kernel_spmd


def _ensure_ntff_hook():
    """This image's antenv lacks axon_hooks; synthesize it so trace=True
    (NTFF profiling) works through run_bass_kernel_spmd."""
    import types

    try:
        from antenv import axon_hooks  # noqa: F401
        return
    except ImportError:
        pass
    try:
        import antenv
        from trn_agent_boot.trn_boot import _ntff_profile_via_ctypes

        mod = types.ModuleType("antenv.axon_hooks")
        _hook = [None]
        mod.set_axon_ntff_profile_hook = lambda h: _hook.__setitem__(0, h)
        mod.get_axon_ntff_profile_hook = lambda: _hook[0]
        sys.modules["antenv.axon_hooks"] = mod
        antenv.axon_hooks = mod
        mod.set_axon_ntff_profile_hook(
            _ntff_profile_via_ctypes("/opt/axon/libaxon_pjrt.so"))
    except Exception as e:  # pragma: no cover
        print(f"ntff hook shim failed ({e}); tracing disabled", file=sys.stderr)


_ensure_ntff_hook()

F32 = mybir.dt.float32
BF16 = mybir.dt.bfloat16

N_CORES = 8
NT = 4096          # tokens per core
DIM = 512          # model dim
INNER = 512        # heads * dim_head
HEADS = 8
DH = 64            # dim head
NSUP = NT // 512   # 512-token supertiles
P = 128

SCALE = DH ** -0.5

QK_BUFS = 3
WARM_P0 = int(os.environ.get("KERNEL_WARM_P0", "0"))
WARM_OT = int(os.environ.get("KERNEL_WARM_OT", "0"))
WARM_ATT = int(os.environ.get("KERNEL_WARM_ATT", "0"))
# "gps" = single 4D store per fin tile issued from the gpsimd SWDGE;
# "sync" = v1-style 8 small stores per fin tile from the sync sequencer.
YDMA = os.environ.get("KERNEL_YDMA", "gps")


def build_kernel(zero_bias=False):
    nc = bacc.Bacc("TRN2", target_bir_lowering=False, debug=False)

    # x is pre-transposed + bf16-cast + token-grouped on HOST (same as the
    # weights): chunk (T, c) = xtg[T*4+c] is a CONTIGUOUS [128, 512] block
    # (feature chunk c, supertile T) so each load is one linear 128KB run.
    xtg = nc.dram_tensor("xtg", [NSUP * 4, P, 512], BF16,
                         kind="ExternalInput").ap()
    # per-tile contiguous weight blocks (chunk c on rows); wqkc is further
    # blocked per F-column-block so the first GEMM group's weights arrive
    # within ~2us: wqkc[c, F] = [128, 128] contiguous.
    wqkc = nc.dram_tensor("wqkc", [4, 8, P, P], BF16, kind="ExternalInput").ap()
    wvc = nc.dram_tensor("wvc", [4, P, 512], BF16, kind="ExternalInput").ap()
    woc = nc.dram_tensor("woc", [4, P, 512], BF16, kind="ExternalInput").ap()
    biasb = nc.dram_tensor("biasb", [P, DIM], F32, kind="ExternalInput").ap()
    permc = nc.dram_tensor("permc", [P, P], BF16, kind="ExternalInput").ap()
    # [8, 8*512]: 16*one-hot(group-of-16) mask rows, tiled across 8 blocks.
    # Loaded into the unused halves of the qk one-head-per-block tiles; the
    # zero rows below/above them are memset on-chip (was 3MB of DMA'd zeros).
    padc = nc.dram_tensor("padc", [8, 8 * 512], BF16, kind="ExternalInput").ap()
    y = nc.dram_tensor("y", [NT, DIM], F32, kind="ExternalOutput").ap()

    # store-side view: the O^T transpose uses a PERMUTATION matrix instead of
    # identity, so fin comes out of GEMM2 with partitions ordered (a, mb, e) =
    # natural y-row order; per (nb, a) a fin tile is one contiguous 64KB run.
    ya = y.rearrange("(nb a rw) f -> nb a (rw f)", nb=2, a=4, rw=512)

    with TileKernel(nc) as tc:
        ctx = tc._ctx
        const = ctx.enter_context(tc.tile_pool(name="const", bufs=1))
        xt_pool = ctx.enter_context(tc.tile_pool(name="xt", bufs=1))
        xload = ctx.enter_context(tc.tile_pool(name="xload", bufs=8))
        xbpool = ctx.enter_context(tc.tile_pool(name="xb", bufs=16))
        v_pool = ctx.enter_context(tc.tile_pool(name="v", bufs=3))
        p_pool = ctx.enter_context(tc.tile_pool(name="p", bufs=4))
        r_pool = ctx.enter_context(tc.tile_pool(name="r", bufs=24))
        o_pool = ctx.enter_context(tc.tile_pool(name="o", bufs=3))
        ot_pool = ctx.enter_context(tc.tile_pool(name="ot", bufs=3))
        f_pool = ctx.enter_context(tc.tile_pool(name="f", bufs=10))
        psum = ctx.enter_context(tc.tile_pool(name="psum", bufs=2, space="PSUM"))

        # --- PE warm-up: ~50 dummy N=128 matmuls on a memset tile, runnable
        # the moment the preamble ends.  Gets the HAM clock gate to 8/8
        # (2.4 GHz) before the first real GEMM group issues. ---
        warmw = const.tile([P, P], BF16, tag="warmw")
        nc.vector.memset(warmw[:], 0.0)
        ps_warm = psum.tile([P, P], F32, name="ps_warm", tag="att", bufs=3)
        for _ in range(100):
            nc.tensor.matmul(ps_warm[:, 0:64], warmw[:], warmw[:, 0:64],
                             start=True, stop=True)

        # --- constants / weights (GEMM1 weights first: they gate GEMM1a(0)).
        # Early loads are split into partition-quarter DMAs round-robined
        # over the three issue paths so they spread across many queues. ---
        # HWDGE only: gpsimd SWDGE pays ~620ns of descriptor-gen per DMA on
        # the gpsimd engine and serializes -- keep it off the critical path.
        ld_engs = [nc.sync, nc.scalar]
        ld_i = [0]

        def fast_load(dst, src, ways=4):
            n = dst.shape[0]
            step = n // ways
            for qq in range(ways):
                ld_engs[ld_i[0] % len(ld_engs)].dma_start(
                    dst[qq * step:(qq + 1) * step], src[qq * step:(qq + 1) * step])
                ld_i[0] += 1

        wqk = []
        wv = []
        wo = []
        for c in range(4):
            t = const.tile([P, 1024], BF16, tag=f"wqk{c}")
            wqk.append(t)

        def load_wqk(F):
            # F-major: each GEMM1a group's 4 weight blocks land together,
            # in the order the groups execute.
            for c in range(4):
                ld_engs[ld_i[0] % len(ld_engs)].dma_start(
                    wqk[c][:, F * P:(F + 1) * P], wqkc[c, F])
                ld_i[0] += 1

        expb = const.tile([P, 1], F32, tag="expb")
        nc.vector.memset(expb[:], -32.0)

        # --- x load: xtg is already transposed/grouped/bf16 (host-prepped);
        # T=0/1 chunks issue right after the GEMM1 weights so supertile 0
        # can start within a few us; the remaining consts and supertiles
        # follow.  Spread across the sync/scalar HWDGE + gpsimd SWDGE
        # queue-sets. ---
        xT = xt_pool.tile([P, 4 * NT], BF16)  # chunk c at cols [c*NT, ...)

        def load_xt(T, ways=1):
            for c in range(4):
                fast_load(
                    xT[:, c * NT + T * 512: c * NT + (T + 1) * 512],
                    xtg[T * 4 + c], ways=ways)

        load_xt(0, ways=2)
        for F in range(8):
            load_wqk(F)

        # qk buffers: even-head blocks put data at rows 0:64 + mask at 64:72
        # and the S matmuls contract only K=72 rows, so rows 72:128 stay
        # uninitialized-but-unread (no memset).  Odd-head blocks have data at
        # 64:128 + mask at 0:8 with K=128, so rows 8:64 must be zeroed once.
        # Mask DMAs are tiny (64KB) and emitted early: attention(0) needs
        # them at ~15us and they must not queue behind bulk loads.
        qk_tiles = []
        for i in range(QK_BUFS):
            t = const.tile([P, 16 * 512], BF16, tag=f"qkbuf{i}")
            # all on gpsimd: V must be free for the first GEMM1a evictions
            nc.gpsimd.memset(t[0:64, 8 * 512:16 * 512], 0.0)
            nc.scalar.dma_start(t[64:72, 0:8 * 512], padc[:])
            nc.sync.dma_start(t[0:8, 8 * 512:16 * 512], padc[:])
            qk_tiles.append(t)

        for c in range(4):
            t = const.tile([P, 512], BF16, tag=f"wv{c}")
            fast_load(t[:], wvc[c], ways=2)
            wv.append(t)
        load_xt(1, ways=2)

        for c in range(4):
            t = const.tile([P, 512], BF16, tag=f"wo{c}")
            nc.scalar.dma_start(t[:], woc[c])
            wo.append(t)
        permb = const.tile([P, P], BF16, tag="permb")
        nc.sync.dma_start(permb[:], permc[:])
        bias = const.tile([P, DIM], F32, tag="bias")
        nc.scalar.dma_start(bias[:], biasb[:])
        # Fixed qk double... triple-buffer with pad halves written ONCE.
        # (Fixed const tiles instead of a rotating pool so the preloaded
        # mask+zero rows deterministically persist across supertiles.)
        # Block layout (even/odd-contiguous so the two pad DMAs are plain
        # contiguous writes): even heads h -> k block h/2, q block 4+h/2
        # (data rows 0..64, pad rows 64..128); odd heads h -> k block 8+h/2,
        # q block 12+h/2 (data rows 64..128, pad rows 0..64).
        # bulk loads stay OFF the scalar (ACT) queue: 20+ queued DMA-issue
        # ops fill the ring and head-block ACT's instruction FIFO, stalling
        # the GEMM1a evictions behind them.
        xt_rest_engs = [nc.sync, nc.gpsimd]
        for T in range(2, NSUP):
            for c in range(4):
                xt_rest_engs[(T * 4 + c) % 2].dma_start(
                    xT[:, c * NT + T * 512: c * NT + (T + 1) * 512],
                    xtg[T * 4 + c],
                )

        st_engs = [nc.gpsimd, nc.sync, nc.scalar]

        def emit_g2_tt(gT, g_ot, tt):
            ps = psum.tile([P, 512], F32, name="ps_g2", tag="aux", bufs=2)
            for c in range(4):
                nc.tensor.matmul(
                    ps[:],
                    g_ot[:, tt * 512 + c * P: tt * 512 + (c + 1) * P],
                    wo[c][:],
                    start=(c == 0),
                    stop=(c == 3),
                )
            fin = f_pool.tile([P, DIM], F32, name="fin")
            if zero_bias:
                # b_out == 0: plain eviction on ACT keeps the busier DVE free
                nc.scalar.copy(fin[:], ps[:])
            else:
                nc.vector.tensor_tensor(
                    fin[:], ps[:], bias[:], op=mybir.AluOpType.add)
            t_idx = gT * 4 + tt
            nb, ms = t_idx // 16, 8 * (t_idx % 16)
            # fin partitions are (a, mb, e) = natural order; per a the 32
            # rows land in one contiguous 64KB run of y.  Four separate
            # DMAs on rotating queue-sets so the drain parallelizes.
            for a in range(4):
                st_engs[(t_idx * 4 + a) % 3].dma_start(
                    ya[nb, a, ms * 2048: ms * 2048 + 16384],
                    fin[a * 32:(a + 1) * 32, :],
                )

        def emit_g2(gT, g_ot):
            for tt in range(4):
                emit_g2_tt(gT, g_ot, tt)

        prev_ot = None
        # --- main loop over 512-token supertiles ---
        # emit_g2(T-1) is emitted FIRST in iteration T: its DVE bias-adds
        # then precede iteration T's evictions in the engine FIFOs, and its
        # matmuls are immediately runnable -- emitting it after T's body put
        # a blocked fin-bias at the DVE queue head and convoyed the PE.
        for T in range(NSUP):
            if T > 0:
                emit_g2(T - 1, prev_ot)
            # qk one-head-per-block layout: block b (512 cols) holds head
            # features in one 64-row half, preloaded mask+zero rows in the
            # other.  k head h -> block h; q head h -> block 8+h.
            # Even h: data rows 0..64 (pad rows 64..128);
            # odd  h: data rows 64..128 (pad rows 0..64).
            qk_sb = qk_tiles[T % QK_BUFS]

            # GEMM1a: qkT [1024_fo, 512_tok] -> bf16, one head per block.
            # BOTH half-evictions of a group go to ONE engine (alternating
            # per F): concurrent engines then always read different PSUM
            # banks (same-bank dual reads serialize ~700ns).
            for F in range(8):
                ps = psum.tile([P, 512], F32, name="ps_g1a", tag="g1a", bufs=3)
                for c in range(4):
                    nc.tensor.matmul(
                        ps[:],
                        wqk[c][:, F * P:(F + 1) * P],
                        xT[:, c * NT + T * 512: c * NT + (T + 1) * 512],
                        start=(c == 0),
                        stop=(c == 3),
                    )
                # F 0..3 = q heads (2F, 2F+1); F 4..7 = k heads (2(F-4), ...)
                if F < 4:
                    be, bo = 4 + F, 12 + F
                else:
                    be, bo = F - 4, 8 + (F - 4)
                dst_e = qk_sb[0:64, be * 512:(be + 1) * 512]
                dst_o = qk_sb[64:128, bo * 512:(bo + 1) * 512]
                if F % 2 == 0:
                    nc.vector.tensor_copy(dst_e, ps[0:64, :])
                    nc.vector.tensor_copy(dst_o, ps[64:128, :])
                else:
                    nc.scalar.copy(dst_e, ps[0:64, :])
                    nc.scalar.copy(dst_o, ps[64:128, :])

            # GEMM1b: v [512_tok, 512_fo] -> bf16 (token-major), 66-stride
            # per head with a ones-column at offset 64 (fused denominator)
            v_sb = v_pool.tile([P, 4 * 528], BF16, name="v_sb")
            vview = v_sb[:].rearrange("p (tt h e) -> p tt h e", tt=4, h=8)
            nc.vector.memset(vview[:, :, :, 64:65], 1.0)
            for tt in range(4):
                ps = psum.tile([P, 512], F32, name="ps_g1b", tag="aux", bufs=2)
                for c in range(4):
                    nc.tensor.matmul(
                        ps[:],
                        xT[:, c * NT + T * 512 + tt * P:
                           c * NT + T * 512 + (tt + 1) * P],
                        wv[c][:],
                        start=(c == 0),
                        stop=(c == 3),
                    )
                vdst = vview[:, tt, :, 0:64]
                nc.vector.tensor_copy(
                    vdst, ps[:].rearrange("p (h d) -> p h d", h=8))

            # attention: tt outer, head-groups of 4 inner
            o_sb = o_pool.tile([P, 4 * 512], BF16)
            ot_sb = ot_pool.tile([P, 4 * 512], BF16)
            for tt in range(4):
                psAs = []
                for hg in range(2):
                    # one PSUM bank per (tt, hg) chain: S writes [P,512],
                    # exp consumes it, then O reuses cols 0:288 of the SAME
                    # bank (frees banks for deeper GEMM pipelining).
                    psA = psum.tile([P, 512], F32, name="psA", tag="att",
                                    bufs=3)
                    psAs.append(psA)
                    for hh in range(4):
                        h = hg * 4 + hh
                        kb = h // 2 if h % 2 == 0 else 8 + h // 2
                        qb = 4 + h // 2 if h % 2 == 0 else 12 + h // 2
                        # even heads: data rows 0:64 + mask 64:72 -> K=72
                        # (rows 72:128 never initialized); odd heads: K=128.
                        kp = 72 if h % 2 == 0 else P
                        ksl = qk_sb[0:kp, kb * 512 + tt * P:
                                    kb * 512 + (tt + 1) * P]
                        qsl = qk_sb[0:kp, qb * 512 + tt * P:
                                    qb * 512 + (tt + 1) * P]
                        nc.tensor.matmul(
                            psA[:, hh * P:(hh + 1) * P], ksl, qsl,
                            start=True, stop=True,
                        )
                    p_sb = p_pool.tile([P, 512], BF16)
                    nc.scalar.activation(
                        p_sb[:], psA[:],
                        mybir.ActivationFunctionType.Exp,
                        bias=expb[:], scale=SCALE,
                    )
                    for hh in range(4):
                        h = hg * 4 + hh
                        psl = p_sb[:, hh * P:(hh + 1) * P]
                        nc.tensor.matmul(
                            psA[:, hh * 72: hh * 72 + 65],
                            psl,
                            v_sb[:, tt * 528 + h * 66:
                                 tt * 528 + h * 66 + 65],
                            start=True, stop=True,
                        )

                # normalize + evict O (token-major bf16)
                for hg in range(2):
                    rc = r_pool.tile([P, 4], F32, name="rc")
                    dsl = psAs[hg][:, 0:288].rearrange("p (h e) -> p h e", h=4)
                    nc.vector.reciprocal(rc[:], dsl[:, :, 64:65].squeeze(2))
                    dst = o_sb[:, tt * 512 + hg * 256:
                               tt * 512 + (hg + 1) * 256].rearrange(
                        "p (h d) -> p h d", h=4)
                    rbc = rc[:].unsqueeze(2).broadcast_to([P, 4, DH])
                    nc.vector.tensor_tensor(
                        dst, dsl[:, :, 0:64], rbc, op=mybir.AluOpType.mult)

                # O^T for this tt (needs only this tt's normalize): regular
                # bf16 matmul lhsT.T @ Perm -- Perm reorders token columns
                # (mb,a,e)->(a,mb,e) so GEMM2's output partitions are already
                # in natural y-row order.  Emitting per-tt spreads the
                # transpose matmuls and their evictions through the attention
                # phase instead of bunching them at the supertile boundary.
                ps_ot = psum.tile([P, 512], F32, name="ps_ot", tag="aux",
                                  bufs=2)
                for c in range(4):
                    nc.tensor.matmul(
                        ps_ot[:, c * P:(c + 1) * P],
                        o_sb[:, tt * 512 + c * P: tt * 512 + (c + 1) * P],
                        permb[:],
                        start=True, stop=True,
                    )
                if tt % 2 == 0:
                    nc.scalar.copy(
                        ot_sb[:, tt * 512:(tt + 1) * 512], ps_ot[:])
                else:
                    nc.vector.tensor_copy(
                        ot_sb[:, tt * 512:(tt + 1) * 512], ps_ot[:])
                if T == NSUP - 1:
                    # final supertile: emit its GEMM2 per-tt right behind the
                    # ot eviction so the tail is one tt deep, not four.
                    emit_g2_tt(T, ot_sb, tt)

            prev_ot = ot_sb

    nc.compile()
    return nc


class TileKernel:
    """TileContext wrapper that also owns an ExitStack for pools."""

    def __init__(self, nc):
        self.nc = nc
        self._tc = tile.TileContext(nc)
        self._ctx = ExitStack()

    def __enter__(self):
        tc = self._tc.__enter__()
        tc._ctx = self._ctx
        return tc

    def __exit__(self, *exc):
        self._ctx.close()
        return self._tc.__exit__(*exc)


def _host_inputs(x, w_qkv, w_out, b_out):
    # x -> xT in grouped token order, bf16: token (nb, a, mb, e) lands at
    # grouped col nb*2048 + mb*16 + a*4 + e; feature f on rows.
    b = x.shape[0]
    xtg = (
        x.reshape(b, 2, 4, 128, 4, DIM)          # b, nb, a, mb, e, f
        .transpose(0, 5, 1, 3, 2, 4)             # b, f, nb, mb, a, e
        .reshape(b, DIM, NT)
        .astype(ml_dtypes.bfloat16)
    )
    # chunk (T, c) contiguous: [b, 32, 128, 512] with index T*4+c
    xtg = np.ascontiguousarray(
        xtg.reshape(b, 4, P, NSUP, 512).transpose(0, 3, 1, 2, 4)
        .reshape(b, NSUP * 4, P, 512))
    wqkvT = np.ascontiguousarray(w_qkv.T).astype(ml_dtypes.bfloat16)
    woutT = np.ascontiguousarray(w_out.T).astype(ml_dtypes.bfloat16)
    wqkc = np.ascontiguousarray(
        wqkvT[:, 0:1024].reshape(4, P, 8, P).transpose(0, 2, 1, 3))
    wvc = np.ascontiguousarray(
        wqkvT[:, 1024:1536].reshape(4, P, 512))
    woc = np.ascontiguousarray(woutT.reshape(4, P, 512))
    biasb = np.ascontiguousarray(
        np.broadcast_to(b_out.astype(np.float32), (P, DIM)))
    # permutation: out col j = a*32+mb*4+e reads token p = mb*16+a*4+e
    permc = np.zeros((P, P), dtype=np.float32)
    for mb in range(8):
        for a in range(4):
            for e in range(4):
                permc[mb * 16 + a * 4 + e, a * 32 + mb * 4 + e] = 1.0
    permc = permc.astype(ml_dtypes.bfloat16)
    # pad block: 16*one-hot(group-of-16 of col%128) mask rows only
    pad1 = np.zeros((8, 512), dtype=np.float32)
    for g in range(8):
        for rep in range(4):
            pad1[g, rep * 128 + g * 16: rep * 128 + (g + 1) * 16] = 16.0
    padc = np.tile(pad1, (1, 8)).astype(ml_dtypes.bfloat16)
    shared = {
        "wqkc": wqkc, "wvc": wvc, "woc": woc, "biasb": biasb,
        "permc": np.ascontiguousarray(permc),
        "padc": np.ascontiguousarray(padc),
    }
    return [
        {"xtg": np.ascontiguousarray(xtg[i]), **shared}
        for i in range(N_CORES)
    ]


_NC_CACHE = {}


def _get_nc(zero_bias):
    key = ("nc", zero_bias)
    if key not in _NC_CACHE:
        _NC_CACHE[key] = build_kernel(zero_bias=zero_bias)
    return _NC_CACHE[key]


def kernel(x, w_qkv, w_out, b_out, _trace=False, _trace_kwargs=None):
    nc = _get_nc(bool(np.all(np.asarray(b_out) == 0.0)))
    in_maps = _host_inputs(x, w_qkv, w_out, b_out)
    kw = {}
    if _trace:
        kw = dict(trace=True, **(_trace_kwargs or {}))
    res = run_bass_kernel_spmd(nc, in_maps, core_ids=list(range(N_CORES)), **kw)
    out = np.stack([res.results[i]["y"] for i in range(N_CORES)], axis=0)
    if _trace:
        kernel.last_results = res
    return out

